# revision 16
# baseline (speedup 1.0000x reference)
"""Bass/Trainium2 kernel for nn_DefenseTrajectoryPredictor.

Model: LayerNorm(158) -> LSTM(158->128) -> LSTM(128->128) -> Linear(128->64)
       -> ReLU -> Linear(64->22), over x[1024, 250, 158].

Sharding: pure data parallel, batch 1024 -> 128 rows per each of 8 cores.

Device layout ("hidden-major"): gate tiles are [gate_hidden(partition), batch
(free)], so the LSTM state h is produced directly in the layout the next
step's matmul consumes (lhsT = W slices, rhs = h). LayerNorm is folded into
the layer-0 input matmul via two augmented contraction rows; the per-sample
rsqrt scale is applied to a host-pre-transposed bf16 copy of x on-chip.
"""

import sys

for _p in ("/opt/trn_rl_repo",):
    if _p not in sys.path:
        sys.path.insert(0, _p)

import numpy as np

import concourse.bass as bass
import concourse.bacc as bacc
import concourse.tile as tile
from concourse import mybir
from concourse.bass_utils import run_bass_kernel_spmd

F32 = mybir.dt.float32
BF16 = mybir.dt.bfloat16
NP_BF16 = mybir.dt.np(BF16)
AF = mybir.ActivationFunctionType
ALU = mybir.AluOpType
AX = mybir.AxisListType

B, T, D, H, PROJ, OUT = 1024, 250, 158, 128, 64, 22
G4 = 4 * H
NC_N = 8
BL = B // NC_N            # 128 batch rows per core
DLO = D - H               # 30
KLO = 32                  # lo contraction rows: 30 x + a-row + ones-row
LN_EPS = 1e-5

# gate permutation: torch order (i,f,g,o) -> device order (i,f,o,g)
_PERM = np.concatenate([np.arange(0, 128), np.arange(128, 256),
                        np.arange(384, 512), np.arange(256, 384)])

_COMPILED = {}
_DEBUG = False


def _build_program(b1_nonzero: bool):
    nc = bacc.Bacc("TRN2", target_bir_lowering=False, debug=False,
                   num_devices=NC_N)

    dt_in = {}

    def din(name, shape, dt):
        dt_in[name] = nc.dram_tensor(name, shape, dt, kind="ExternalInput")
        return dt_in[name]

    xnat = din("xnat", [BL, T * D], F32)
    xthi = din("xthi", [H, T * BL], BF16)
    xtlo = din("xtlo", [KLO, T * BL], BF16)
    a_hi = din("a_hi", [H, G4], BF16)
    a_lo = din("a_lo", [KLO, G4], BF16)
    whh0 = din("whh0", [H, G4], BF16)
    wih1 = din("wih1", [H, G4], BF16)
    whh1 = din("whh1", [H, G4], BF16)
    wp1 = din("wp1", [H, PROJ], BF16)
    wp2 = din("wp2", [PROJ, OUT], BF16)
    bp1 = din("bp1", [PROJ, 1], F32)
    bp2b8 = din("bp2b8", [BL, 8 * OUT], F32)
    ident = din("ident", [H, H], F32)
    if b1_nonzero:
        b1r = din("b1r", [1, G4], BF16)

    out_d = nc.dram_tensor("out", [BL, T * OUT], F32, kind="ExternalOutput")
    if _DEBUG:
        dbg_r = nc.dram_tensor("dbg_r", [BL, 256], F32, kind="ExternalOutput")
        dbg_a = nc.dram_tensor("dbg_a", [BL, 256], F32, kind="ExternalOutput")
        dbg_xn = nc.dram_tensor("dbg_xn", [H, 512], BF16,
                                kind="ExternalOutput")
        dbg_xl = nc.dram_tensor("dbg_xl", [KLO, 512], BF16,
                                kind="ExternalOutput")
        dbg_pg = nc.dram_tensor("dbg_pg", [BL, G4], F32,
                                kind="ExternalOutput")
        dbg_h = nc.dram_tensor("dbg_h", [BL, 2 * H], BF16,
                               kind="ExternalOutput")
        dbg_c = nc.dram_tensor("dbg_c", [BL, 2 * H], F32,
                               kind="ExternalOutput")

    with tile.TileContext(nc) as tc:
        with tc.tile_pool(name="const", bufs=1) as const:
            # ---- persistent weights / state ------------------------------
            c_ahi = const.tile([H, G4], BF16)
            c_alo = const.tile([KLO, G4], BF16)
            c_whh0 = const.tile([H, G4], BF16)
            c_wih1 = const.tile([H, G4], BF16)
            c_whh1 = const.tile([H, G4], BF16)
            c_wp1 = const.tile([H, PROJ], BF16)
            c_wp2 = const.tile([PROJ, OUT], BF16)
            c_bp1 = const.tile([PROJ, 1], F32)
            c_bp2 = const.tile([BL, 8 * OUT], F32)
            c_id = const.tile([H, H], F32)
            for dst, src in [(c_ahi, a_hi), (c_alo, a_lo), (c_whh0, whh0),
                             (c_wih1, wih1), (c_whh1, whh1), (c_wp1, wp1),
                             (c_wp2, wp2), (c_bp1, bp1), (c_bp2, bp2b8),
                             (c_id, ident)]:
                nc.sync.dma_start(out=dst[:], in_=src.ap())
            if b1_nonzero:
                c_b1 = const.tile([1, G4], BF16)
                nc.sync.dma_start(out=c_b1[:], in_=b1r.ap())
                c_ones1 = const.tile([1, BL], BF16)
                nc.vector.memset(c_ones1[:], 1.0)

            eps_t = const.tile([BL, 1], F32)
            nc.vector.memset(eps_t[:], LN_EPS)

            h_pair = const.tile([BL, 2, H], BF16)   # [:,0,:]=h0  [:,1,:]=h1
            c_pair = const.tile([BL, 2, H], F32)
            nc.vector.memset(h_pair[:], 0.0)
            nc.vector.memset(c_pair[:], 0.0)

            sums_x = const.tile([BL, 256], F32)
            sums_q = const.tile([BL, 256], F32)
            mu_t = const.tile([BL, 256], F32)
            rr_t = const.tile([BL, 256], F32)
            rT = const.tile([128, 2, BL], BF16)     # r transposed: [t%128, t//128, b]
            aT = const.tile([128, 2, BL], BF16)     # -r*mu transposed

            # ---- phase 1: LN statistics (natural layout) -----------------
            xnat_ap = xnat.ap()
            with tc.tile_pool(name="stat", bufs=3) as statp:
                c0 = 0
                while c0 < T:
                    cs = min(8, T - c0)
                    xt = statp.tile([BL, 8, D], F32, tag="stat_x")
                    src = xnat_ap[:, c0 * D:(c0 + cs) * D].rearrange(
                        "p (t d) -> p t d", d=D)
                    nc.sync.dma_start(out=xt[:, 0:cs, :], in_=src)
                    sq = statp.tile([BL, 8, D], F32, tag="stat_q")
                    nc.scalar.square(sq[:, 0:cs, :], xt[:, 0:cs, :])
                    nc.vector.tensor_reduce(sums_x[:, c0:c0 + cs],
                                            xt[:, 0:cs, :], axis=AX.X,
                                            op=ALU.add)
                    nc.vector.tensor_reduce(sums_q[:, c0:c0 + cs],
                                            sq[:, 0:cs, :], axis=AX.X,
                                            op=ALU.add)
                    c0 += cs

                # mu = sum/D ; var = sumsq/D - mu^2 ; r = 1/sqrt(var+eps)
                nc.vector.tensor_scalar_mul(mu_t[:, 0:T], sums_x[:, 0:T],
                                            1.0 / D)
                nc.vector.tensor_scalar_mul(sums_q[:, 0:T], sums_q[:, 0:T],
                                            1.0 / D)
                nc.vector.tensor_mul(sums_x[:, 0:T], mu_t[:, 0:T],
                                     mu_t[:, 0:T])          # mu^2
                nc.vector.tensor_sub(sums_q[:, 0:T], sums_q[:, 0:T],
                                     sums_x[:, 0:T])        # var
                nc.scalar.activation(sums_q[:, 0:T], sums_q[:, 0:T], AF.Sqrt,
                                     bias=eps_t[:, 0:1], scale=1.0)
                nc.vector.reciprocal(rr_t[:, 0:T], sums_q[:, 0:T])  # r
                nc.vector.tensor_mul(mu_t[:, 0:T], mu_t[:, 0:T],
                                     rr_t[:, 0:T])
                nc.vector.tensor_scalar_mul(mu_t[:, 0:T], mu_t[:, 0:T],
                                            -1.0)           # a = -mu*r

                # transpose r, a into [t, b] orientation via PE
                with tc.tile_pool(name="pstat", bufs=2, space="PSUM") as pst:
                    for (srct, dstt) in ((rr_t, rT), (mu_t, aT)):
                        for j in range(2):
                            w = 128 if j == 0 else T - 128
                            ps = pst.tile([128, 128], F32, tag="pstat_t")
                            nc.tensor.transpose(ps[0:w, :],
                                                srct[:, j * 128:j * 128 + w],
                                                c_id[:])
                            nc.vector.tensor_copy(dstt[0:w, j, :], ps[0:w, :])

            # ---- phase 2: fused LSTM x2 + MLP ----------------------------
            xthi_ap = xthi.ap()
            xtlo_ap = xtlo.ap()
            out_ap = out_d.ap()

            with tc.tile_pool(name="dramst", bufs=1, space="DRAM") as dramst, \
                 tc.tile_pool(name="work", bufs=3) as work, \
                 tc.tile_pool(name="cell", bufs=3) as cell, \
                 tc.tile_pool(name="mlpp", bufs=3) as mlpp, \
                 tc.tile_pool(name="pg", bufs=2, space="PSUM") as pgp, \
                 tc.tile_pool(name="pp", bufs=2, space="PSUM") as ppp, \
                 tc.tile_pool(name="po", bufs=2, space="PSUM") as pop:

                rT_d = dramst.tile([128, 2, BL], BF16)
                aT_d = dramst.tile([128, 2, BL], BF16)
                nc.sync.dma_start(out=rT_d[:], in_=rT[:])
                nc.sync.dma_start(out=aT_d[:], in_=aT[:])
                if _DEBUG:
                    nc.sync.dma_start(out=dbg_r.ap(), in_=rr_t[:])
                    nc.sync.dma_start(out=dbg_a.ap(), in_=mu_t[:])

                chunk_tiles = {}

                def chunk_prep(tc0):
                    cs = min(4, T - tc0)
                    xh = work.tile([H, 4 * BL], BF16, tag="ck_xh")
                    xl = work.tile([KLO, 4 * BL], BF16, tag="ck_xl")
                    rb = work.tile([BL, 4 * BL], BF16, tag="ck_rb")
                    xn = work.tile([H, 4 * BL], BF16, tag="ck_xn")
                    w = cs * BL
                    nc.sync.dma_start(out=xh[:, 0:w],
                                      in_=xthi_ap[:, tc0 * BL:tc0 * BL + w])
                    nc.sync.dma_start(out=xl[:, 0:w],
                                      in_=xtlo_ap[:, tc0 * BL:tc0 * BL + w])
                    for k in range(cs):
                        t = tc0 + k
                        j, p = divmod(t, 128)
                        src = rT_d[p, j, :]
                        src_b = bass.AP(tensor=src.tensor, offset=src.offset,
                                        ap=[[0, BL]] + list(src.ap))
                        nc.sync.dma_start(out=rb[:, k * BL:(k + 1) * BL],
                                          in_=src_b)
                        nc.sync.dma_start(out=xl[30:31, k * BL:(k + 1) * BL],
                                          in_=aT_d[p:p + 1, j, :])
                    # xn_hi = x_hi * r ; xlo rows 0:30 scaled in place
                    nc.vector.tensor_mul(xn[:, 0:w], xh[:, 0:w], rb[:, 0:w])
                    nc.vector.tensor_mul(xl[0:DLO, 0:w], xl[0:DLO, 0:w],
                                         rb[0:DLO, 0:w])
                    if _DEBUG and tc0 == 0:
                        nc.sync.dma_start(out=dbg_xn.ap(), in_=xn[:])
                        nc.sync.dma_start(out=dbg_xl.ap(), in_=xl[:])
                    chunk_tiles[tc0 // 4] = (xn, xl)

                def gsl(g):
                    return slice(g * H, (g + 1) * H)

                chunk_prep(0)
                po_t = None
                for t in range(T + 1):
                    if t % 4 == 0 and t + 4 < T:
                        chunk_prep(t + 4)
                    do0 = t < T
                    do1 = t >= 1
                    m1 = t - 1          # layer-1 step this tick
                    mm = t - 2          # mlp step this tick

                    # ---- MLP for step mm (h1_mm written 2 ticks ago) ----
                    if 0 <= mm:
                        r8 = mm % 8
                        if r8 == 0:
                            po_t = pop.tile([BL, 8 * OUT], F32, tag="po")
                        pp_t = ppp.tile([PROJ, BL], F32, tag="pp")
                        nc.tensor.matmul(pp_t[:], c_wp1[:], h_pair[:, 1, :],
                                         start=True, stop=True)
                        prelu = mlpp.tile([PROJ, BL], BF16, tag="prelu")
                        nc.vector.tensor_scalar(prelu[:], pp_t[:],
                                                c_bp1[:, 0:1], 0.0,
                                                op0=ALU.add, op1=ALU.max)
                        nc.tensor.matmul(po_t[:, r8 * OUT:(r8 + 1) * OUT],
                                         prelu[:], c_wp2[:],
                                         start=(r8 == 0), stop=True)
                        if r8 == 7 or mm == T - 1:
                            n8 = r8 + 1
                            m0 = mm - r8
                            osb = mlpp.tile([BL, 8 * OUT], F32, tag="osb")
                            nc.vector.tensor_add(osb[:, 0:n8 * OUT],
                                                 po_t[:, 0:n8 * OUT],
                                                 c_bp2[:, 0:n8 * OUT])
                            nc.sync.dma_start(
                                out=out_ap[:, m0 * OUT:(m0 + n8) * OUT],
                                in_=osb[:, 0:n8 * OUT])

                    # ---- gate matmuls ----
                    pg_t = pgp.tile([BL, 2, G4], F32, tag="pg")
                    if do0:
                        xn, xl = chunk_tiles[t // 4]
                        k = t % 4
                        xnk = xn[:, k * BL:(k + 1) * BL]
                        xlk = xl[:, k * BL:(k + 1) * BL]
                        for g in range(4):
                            nc.tensor.matmul(pg_t[:, 0, gsl(g)],
                                             c_ahi[:, gsl(g)], xnk,
                                             start=(g == 0), stop=False)
                        for g in range(4):
                            nc.tensor.matmul(pg_t[:, 0, gsl(g)],
                                             c_alo[:, gsl(g)], xlk,
                                             start=False, stop=False)
                    if do1:
                        for g in range(4):
                            nc.tensor.matmul(pg_t[:, 1, gsl(g)],
                                             c_whh1[:, gsl(g)],
                                             h_pair[:, 1, :],
                                             start=(g == 0), stop=False)
                        if b1_nonzero:
                            for g in range(4):
                                nc.tensor.matmul(pg_t[:, 1, gsl(g)],
                                                 c_b1[0:1, gsl(g)],
                                                 c_ones1[0:1, :],
                                                 start=False, stop=False)
                    if do0:
                        for g in range(4):
                            nc.tensor.matmul(pg_t[:, 0, gsl(g)],
                                             c_whh0[:, gsl(g)],
                                             h_pair[:, 0, :],
                                             start=False, stop=(g == 3))
                    if do1:
                        for g in range(4):
                            nc.tensor.matmul(pg_t[:, 1, gsl(g)],
                                             c_wih1[:, gsl(g)],
                                             h_pair[:, 0, :],
                                             start=False, stop=(g == 3))

                    if _DEBUG and t == 0:
                        dtmp = cell.tile([BL, G4], F32, tag="dbgpg")
                        nc.vector.tensor_copy(dtmp[:], pg_t[:, 0, :])
                        nc.sync.dma_start(out=dbg_pg.ap(), in_=dtmp[:])

                    # ---- activations + cell update (merged layers) ----
                    if do0 and do1:
                        sl = slice(0, 2)
                    elif do0:
                        sl = slice(0, 1)
                    else:
                        sl = slice(1, 2)
                    sig = cell.tile([BL, 2, 3 * H], BF16, tag="sig")
                    gp = cell.tile([BL, 2, H], BF16, tag="gp")
                    tcp = cell.tile([BL, 2, H], BF16, tag="tcp")
                    igp = cell.tile([BL, 2, H], BF16, tag="igp")
                    fcp = cell.tile([BL, 2, H], F32, tag="fcp")
                    nc.scalar.activation(sig[:, sl, :], pg_t[:, sl, 0:3 * H],
                                         AF.Sigmoid)
                    nc.scalar.activation(gp[:, sl, :], pg_t[:, sl, 3 * H:G4],
                                         AF.Tanh)
                    nc.vector.tensor_mul(igp[:, sl, :], sig[:, sl, 0:H],
                                         gp[:, sl, :])
                    nc.vector.tensor_mul(fcp[:, sl, :], sig[:, sl, H:2 * H],
                                         c_pair[:, sl, :])
                    nc.vector.tensor_add(c_pair[:, sl, :], fcp[:, sl, :],
                                         igp[:, sl, :])
                    nc.scalar.activation(tcp[:, sl, :], c_pair[:, sl, :],
                                         AF.Tanh)
                    nc.vector.tensor_mul(h_pair[:, sl, :],
                                         sig[:, sl, 2 * H:3 * H],
                                         tcp[:, sl, :])
                    if _DEBUG and t == 1:
                        nc.sync.dma_start(out=dbg_h.ap(), in_=h_pair[:])
                        nc.sync.dma_start(out=dbg_c.ap(), in_=c_pair[:])

                # flush mlp for steps T-2, T-1 (m = T-2 handled at t=T, need T-1)
                for mm in (T - 1,):
                    r8 = mm % 8
                    if r8 == 0:
                        po_t = pop.tile([BL, 8 * OUT], F32, tag="po")
                    pp_t = ppp.tile([PROJ, BL], F32, tag="pp")
                    nc.tensor.matmul(pp_t[:], c_wp1[:], h_pair[:, 1, :],
                                     start=True, stop=True)
                    prelu = mlpp.tile([PROJ, BL], BF16, tag="prelu")
                    nc.vector.tensor_scalar(prelu[:], pp_t[:], c_bp1[:, 0:1],
                                            0.0, op0=ALU.add, op1=ALU.max)
                    nc.tensor.matmul(po_t[:, r8 * OUT:(r8 + 1) * OUT],
                                     prelu[:], c_wp2[:], start=(r8 == 0),
                                     stop=True)
                    n8 = r8 + 1
                    m0 = mm - r8
                    osb = mlpp.tile([BL, 8 * OUT], F32, tag="osb")
                    nc.vector.tensor_add(osb[:, 0:n8 * OUT],
                                         po_t[:, 0:n8 * OUT],
                                         c_bp2[:, 0:n8 * OUT])
                    nc.sync.dma_start(out=out_ap[:, m0 * OUT:(m0 + n8) * OUT],
                                      in_=osb[:, 0:n8 * OUT])

    nc.compile()
    return nc


def _get_program(b1_nonzero: bool):
    key = b1_nonzero
    if key not in _COMPILED:
        _COMPILED[key] = _build_program(b1_nonzero)
    return _COMPILED[key]


def _prep_host(x, ln_gamma, ln_beta, W_ih0, W_hh0, b0, W_ih1, W_hh1, b1,
               Wp1, bp1, Wp2, bp2):
    f32 = np.float32
    x = np.asarray(x, f32)
    g = np.asarray(ln_gamma, f32)
    be = np.asarray(ln_beta, f32)
    W_ih0 = np.asarray(W_ih0, f32)[_PERM]
    W_hh0 = np.asarray(W_hh0, f32)[_PERM]
    b0 = np.asarray(b0, f32)[_PERM]
    W_ih1 = np.asarray(W_ih1, f32)[_PERM]
    W_hh1 = np.asarray(W_hh1, f32)[_PERM]
    b1 = np.asarray(b1, f32)[_PERM]
    Wp1 = np.asarray(Wp1, f32)
    bp1v = np.asarray(bp1, f32)
    Wp2 = np.asarray(Wp2, f32)
    bp2v = np.asarray(bp2, f32)

    Wt0 = W_ih0 * g[None, :]                  # [512, 158]
    u0 = W_ih0 @ g                            # [512]
    v0 = W_ih0 @ be + b0                      # [512]

    a_hi = np.ascontiguousarray(Wt0[:, :H].T).astype(NP_BF16)
    a_lo = np.zeros((KLO, G4), f32)
    a_lo[0:DLO] = Wt0[:, H:D].T
    a_lo[DLO] = u0
    a_lo[DLO + 1] = v0
    a_lo = a_lo.astype(NP_BF16)

    shared = {
        "a_hi": a_hi,
        "a_lo": a_lo,
        "whh0": np.ascontiguousarray(W_hh0.T).astype(NP_BF16),
        "wih1": np.ascontiguousarray(W_ih1.T).astype(NP_BF16),
        "whh1": np.ascontiguousarray(W_hh1.T).astype(NP_BF16),
        "wp1": np.ascontiguousarray(Wp1.T).astype(NP_BF16),
        "wp2": np.ascontiguousarray(Wp2.T).astype(NP_BF16),
        "bp1": np.ascontiguousarray(bp1v.reshape(PROJ, 1)),
        "bp2b8": np.ascontiguousarray(
            np.tile(bp2v[None, :], (BL, 8)).astype(f32)),
        "ident": np.eye(H, dtype=f32),
    }
    b1_nonzero = bool(np.any(b1 != 0))
    if b1_nonzero:
        shared["b1r"] = b1.reshape(1, G4).astype(NP_BF16)

    in_maps = []
    for c in range(NC_N):
        xc = x[c * BL:(c + 1) * BL]                       # [128, 250, 158]
        xT = np.ascontiguousarray(xc.transpose(2, 1, 0)).reshape(D, T * BL)
        xtlo = np.zeros((KLO, T * BL), f32)
        xtlo[0:DLO] = xT[H:D]
        xtlo[DLO + 1] = 1.0
        m = dict(shared)
        m["xnat"] = np.ascontiguousarray(xc.reshape(BL, T * D))
        m["xthi"] = np.ascontiguousarray(xT[0:H]).astype(NP_BF16)
        m["xtlo"] = xtlo.astype(NP_BF16)
        in_maps.append(m)
    return in_maps, b1_nonzero


def kernel(**inputs) -> np.ndarray:
    in_maps, b1_nonzero = _prep_host(**inputs)
    nc = _get_program(b1_nonzero)
    res = run_bass_kernel_spmd(nc, in_maps, core_ids=list(range(NC_N)))
    out = np.empty((B, T, OUT), np.float32)
    for c in range(NC_N):
        out[c * BL:(c + 1) * BL] = res.results[c]["out"].reshape(BL, T, OUT)
    return out


if __name__ == "__main__":
    rng = np.random.default_rng(0)
    # quick structural smoke (random weights, compares vs numpy reference)
    pass


# revision 26
# speedup vs baseline: 1.0856x; 1.0856x over previous
"""Bass/Trainium2 kernel for nn_DefenseTrajectoryPredictor.

Model: LayerNorm(158) -> LSTM(158->128) -> LSTM(128->128) -> Linear(128->64)
       -> ReLU -> Linear(64->22), over x[1024, 250, 158].

Sharding: pure data parallel, batch 1024 -> 128 rows per each of 8 cores.

Device layout ("hidden-major"): gate tiles are [gate_hidden(partition), batch
(free)], so the LSTM state h is produced directly in the layout the next
step's matmul consumes (lhsT = W slices, rhs = h).

Key folds (validated in numpy):
 - LayerNorm folded into the layer-0 input matmul: gamma/beta into the
   weights, per-sample (mu, rsqrt) via two augmented contraction rows; the
   rsqrt scale is applied to a host-pre-transposed bf16 copy of x on-chip.
 - tanh(v) = 2*sigmoid(2v) - 1 for both the g-gate and tanh(c): g-gate
   weight rows are pre-doubled so ONE sigmoid covers all 4 gates; the
   2s-1 corrections fuse into scalar_tensor_tensor ops.
 - h is stored halved (h/2 = o*(sigmoid(2c)-0.5)); every consumer of h is
   a matmul, so those weights are pre-doubled on the host.
"""

import sys

for _p in ("/opt/trn_rl_repo",):
    if _p not in sys.path:
        sys.path.insert(0, _p)

import numpy as np

import concourse.bass as bass
import concourse.bacc as bacc
import concourse.tile as tile
from concourse import mybir
from concourse.bass_utils import run_bass_kernel_spmd

F32 = mybir.dt.float32
BF16 = mybir.dt.bfloat16
FP16 = mybir.dt.float16
NP_BF16 = mybir.dt.np(BF16)
AF = mybir.ActivationFunctionType
ALU = mybir.AluOpType
AX = mybir.AxisListType

B, T, D, H, PROJ, OUT = 1024, 250, 158, 128, 64, 22
G4 = 4 * H
NC_N = 8
BL = B // NC_N            # 128 batch rows per core
DLO = D - H               # 30
KLO = 32                  # lo rows: 30 x + a-row + pad
LN_EPS = 1e-5

# gate permutation: torch order (i,f,g,o) -> device order (i,f,o,g)
_PERM = np.concatenate([np.arange(0, 128), np.arange(128, 256),
                        np.arange(384, 512), np.arange(256, 384)])
_GS = slice(384, 512)     # g-gate slot after permutation

_COMPILED = {}
_DEBUG = False


def _build_program(b1_nonzero: bool, v0_nonzero: bool):
    nc = bacc.Bacc("TRN2", target_bir_lowering=False, debug=False,
                   num_devices=NC_N)

    def din(name, shape, dt):
        return nc.dram_tensor(name, shape, dt, kind="ExternalInput")

    xnat = din("xnat", [BL, T * D], F32)
    xthi = din("xthi", [H, T * BL], BF16)
    xtlo4 = din("xtlo4", [KLO, T * BL], BF16)
    a_hi = din("a_hi", [H, G4], BF16)
    a_lo4 = din("a_lo4", [KLO, G4], BF16)
    whh0 = din("whh0", [H, G4], BF16)
    wih1 = din("wih1", [H, G4], BF16)
    whh1 = din("whh1", [H, G4], BF16)
    wp1 = din("wp1", [H, PROJ], BF16)
    wp2 = din("wp2", [PROJ, OUT], BF16)
    bp1 = din("bp1", [PROJ, 1], F32)
    bp2b8 = din("bp2b8", [BL, 8 * OUT], F32)
    ident = din("ident", [H, H], F32)
    if b1_nonzero:
        b1r = din("b1r", [1, G4], BF16)
    if v0_nonzero:
        v0r = din("v0r", [1, G4], BF16)

    out_d = nc.dram_tensor("out", [BL, T * OUT], F32, kind="ExternalOutput")
    if _DEBUG:
        dbg_r = nc.dram_tensor("dbg_r", [BL, 256], F32, kind="ExternalOutput")
        dbg_a = nc.dram_tensor("dbg_a", [BL, 256], F32, kind="ExternalOutput")
        dbg_pg = nc.dram_tensor("dbg_pg", [BL, G4], F32,
                                kind="ExternalOutput")
        dbg_h = nc.dram_tensor("dbg_h", [BL, 2 * H], BF16,
                               kind="ExternalOutput")
        dbg_c = nc.dram_tensor("dbg_c", [BL, 2 * H], F32,
                               kind="ExternalOutput")

    with tile.TileContext(nc) as tc:
        with tc.tile_pool(name="const", bufs=1) as const:
            # ---- persistent weights / state ------------------------------
            c_ahi = const.tile([H, G4], BF16)
            c_alo = const.tile([KLO, G4], BF16)
            c_whh0 = const.tile([H, G4], BF16)
            c_wih1 = const.tile([H, G4], BF16)
            c_whh1 = const.tile([H, G4], BF16)
            c_wp1 = const.tile([H, PROJ], BF16)
            c_wp2 = const.tile([PROJ, OUT], BF16)
            c_bp1 = const.tile([PROJ, 1], F32)
            c_bp2 = const.tile([BL, 8 * OUT], F32)
            c_id = const.tile([H, H], F32)
            loads = [(c_ahi, a_hi), (c_alo, a_lo4), (c_whh0, whh0),
                     (c_wih1, wih1), (c_whh1, whh1), (c_wp1, wp1),
                     (c_wp2, wp2), (c_bp1, bp1), (c_bp2, bp2b8),
                     (c_id, ident)]
            if b1_nonzero:
                c_b1 = const.tile([1, G4], BF16)
                loads.append((c_b1, b1r))
            if v0_nonzero:
                c_v0 = const.tile([1, G4], BF16)
                loads.append((c_v0, v0r))
            for dst, src in loads:
                nc.sync.dma_start(out=dst[:], in_=src.ap())
            if b1_nonzero or v0_nonzero:
                c_ones1 = const.tile([1, BL], BF16)
                nc.vector.memset(c_ones1[:], 1.0)

            eps_t = const.tile([BL, 1], F32)
            nc.vector.memset(eps_t[:], LN_EPS)

            h_pair = const.tile([BL, 2, H], BF16)   # [:,0,:]=h0/2 [:,1,:]=h1/2
            c_pair = const.tile([BL, 2, H], F32)
            nc.vector.memset(h_pair[:], 0.0)
            nc.vector.memset(c_pair[:], 0.0)

            sums_x = const.tile([BL, 256], F32)
            sums_q = const.tile([BL, 256], F32)
            mu_t = const.tile([BL, 256], F32)
            rr_t = const.tile([BL, 256], F32)
            rT = const.tile([128, 2, BL], BF16)     # [t%128, t//128, b]
            aT = const.tile([128, 2, BL], BF16)
            nc.vector.memset(rT[:], 0.0)
            nc.vector.memset(aT[:], 0.0)

            # ---- phase 1: LN statistics (natural layout) -----------------
            xnat_ap = xnat.ap()
            with tc.tile_pool(name="stat", bufs=3) as statp:
                c0 = 0
                while c0 < T:
                    cs = min(8, T - c0)
                    xt = statp.tile([BL, 8, D], F32, tag="stat_x")
                    src = xnat_ap[:, c0 * D:(c0 + cs) * D].rearrange(
                        "p (t d) -> p t d", d=D)
                    nc.sync.dma_start(out=xt[:, 0:cs, :], in_=src)
                    sq = statp.tile([BL, 8, D], F32, tag="stat_q")
                    nc.scalar.square(sq[:, 0:cs, :], xt[:, 0:cs, :])
                    nc.vector.tensor_reduce(sums_x[:, c0:c0 + cs],
                                            xt[:, 0:cs, :], axis=AX.X,
                                            op=ALU.add)
                    nc.vector.tensor_reduce(sums_q[:, c0:c0 + cs],
                                            sq[:, 0:cs, :], axis=AX.X,
                                            op=ALU.add)
                    c0 += cs

                # mu = sum/D ; var = sumsq/D - mu^2 ; r = 1/sqrt(var+eps)
                nc.vector.tensor_scalar_mul(mu_t[:, 0:T], sums_x[:, 0:T],
                                            1.0 / D)
                nc.vector.tensor_scalar_mul(sums_q[:, 0:T], sums_q[:, 0:T],
                                            1.0 / D)
                nc.vector.tensor_mul(sums_x[:, 0:T], mu_t[:, 0:T],
                                     mu_t[:, 0:T])          # mu^2
                nc.vector.tensor_sub(sums_q[:, 0:T], sums_q[:, 0:T],
                                     sums_x[:, 0:T])        # var
                nc.scalar.activation(sums_q[:, 0:T], sums_q[:, 0:T], AF.Sqrt,
                                     bias=eps_t[:, 0:1], scale=1.0)
                nc.vector.reciprocal(rr_t[:, 0:T], sums_q[:, 0:T])  # r
                nc.vector.tensor_mul(mu_t[:, 0:T], mu_t[:, 0:T],
                                     rr_t[:, 0:T])
                nc.vector.tensor_scalar_mul(mu_t[:, 0:T], mu_t[:, 0:T],
                                            -1.0)           # a = -mu*r

                # transpose r, a into [t, b] orientation via PE
                with tc.tile_pool(name="pstat", bufs=2, space="PSUM") as pst:
                    for (srct, dstt) in ((rr_t, rT), (mu_t, aT)):
                        for j in range(2):
                            w = 128 if j == 0 else T - 128
                            ps = pst.tile([128, 128], F32, tag="pstat_t")
                            nc.tensor.transpose(ps[0:w, :],
                                                srct[:, j * 128:j * 128 + w],
                                                c_id[:])
                            nc.vector.tensor_copy(dstt[0:w, j, :], ps[0:w, :])

            # ---- phase 2: fused LSTM x2 + MLP ----------------------------
            xthi_ap = xthi.ap()
            xtlo_ap = xtlo4.ap()
            out_ap = out_d.ap()

            with tc.tile_pool(name="dramst", bufs=1, space="DRAM") as dramst, \
                 tc.tile_pool(name="work", bufs=3) as work, \
                 tc.tile_pool(name="cell", bufs=3) as cell, \
                 tc.tile_pool(name="mlpp", bufs=3) as mlpp, \
                 tc.tile_pool(name="pg", bufs=3, space="PSUM") as pgp, \
                 tc.tile_pool(name="pp", bufs=1, space="PSUM") as ppp, \
                 tc.tile_pool(name="po", bufs=1, space="PSUM") as pop:

                rT_d = dramst.tile([128, 2, BL], BF16)
                aT_d = dramst.tile([128, 2, BL], BF16)
                nc.sync.dma_start(out=rT_d[:], in_=rT[:])
                nc.sync.dma_start(out=aT_d[:], in_=aT[:])
                if _DEBUG:
                    nc.sync.dma_start(out=dbg_r.ap(), in_=rr_t[:])
                    nc.sync.dma_start(out=dbg_a.ap(), in_=mu_t[:])

                chunk_tiles = {}

                def chunk_prep(tc0):
                    cs = min(4, T - tc0)
                    w = cs * BL
                    xh = work.tile([H, 4 * BL], BF16, tag="ck_xh")
                    xl = work.tile([KLO, 4 * BL], BF16, tag="ck_xl")
                    rb = work.tile([BL, 4 * BL], BF16, tag="ck_rb")
                    xn = work.tile([H, 4 * BL], BF16, tag="ck_xn")
                    nc.sync.dma_start(out=xh[:, 0:w],
                                      in_=xthi_ap[:, tc0 * BL:tc0 * BL + w])
                    nc.sync.dma_start(out=xl[:, 0:w],
                                      in_=xtlo_ap[:, tc0 * BL:tc0 * BL + w])
                    # r broadcast (DRAM src, partition-step 0)
                    rsrc = rT_d[:]
                    asrc = aT_d[:]
                    for k in range(cs):
                        t = tc0 + k
                        j, p = divmod(t, 128)
                        off = p * 256 + j * 128
                        nc.sync.dma_start(
                            out=rb[:, k * BL:(k + 1) * BL],
                            in_=bass.AP(tensor=rsrc.tensor, offset=off,
                                        ap=[[0, BL], [1, BL]]))
                    # scale all rows by r (garbage rows fixed below)
                    nc.vector.tensor_mul(xn[:, 0:w], xh[:, 0:w], rb[:, 0:w])
                    nc.vector.tensor_mul(xl[:, 0:w], xl[:, 0:w],
                                         rb[0:KLO, 0:w])
                    # a row (30): per t
                    for k in range(cs):
                        t = tc0 + k
                        j, p = divmod(t, 128)
                        nc.sync.dma_start(
                            out=xl[30:31, k * BL:(k + 1) * BL],
                            in_=aT_d[p:p + 1, j, :])
                    chunk_tiles[tc0 // 4] = (xn, xl)

                def gsl(g):
                    return slice(g * H, (g + 1) * H)

                pg_tiles = {}

                def emit_xg0(t):
                    # layer-0 input-side matmuls for step t (pipelined early)
                    pg_n = pg_tiles[t]
                    xn, xl = chunk_tiles[t // 4]
                    k = t % 4
                    xnk = xn[:, k * BL:(k + 1) * BL]
                    for g in range(4):
                        nc.tensor.matmul(pg_n[:, 0, gsl(g)],
                                         c_ahi[:, gsl(g)], xnk,
                                         start=(g == 0), stop=False)
                    xlk = xl[:, k * BL:(k + 1) * BL]
                    for g in range(4):
                        nc.tensor.matmul(pg_n[:, 0, gsl(g)],
                                         c_alo[:, gsl(g)], xlk,
                                         start=False, stop=False)
                    if v0_nonzero:
                        for g in range(4):
                            nc.tensor.matmul(pg_n[:, 0, gsl(g)],
                                             c_v0[0:1, gsl(g)],
                                             c_ones1[0:1, :],
                                             start=False, stop=False)

                chunk_prep(0)
                chunk_prep(4)
                pg_tiles[0] = pgp.tile([BL, 2, G4], F32, tag="pg", name="pg0")
                emit_xg0(0)

                po_t = None
                for t in range(T + 1):
                    if t % 4 == 0 and t + 8 < T:
                        chunk_prep(t + 8)
                    do0 = t < T
                    do1 = t >= 1
                    m1 = t - 1          # layer-1 step this tick
                    mm = t - 2          # mlp step this tick
                    pg_t = pg_tiles.pop(t)

                    # ---- MLP for step mm (h1_mm written 2 ticks ago) ----
                    if 0 <= mm:
                        r8 = mm % 8
                        if r8 == 0:
                            po_t = pop.tile([BL, 8 * OUT], F32, tag="po")
                        pp_t = ppp.tile([PROJ, BL], F32, tag="pp")
                        nc.tensor.matmul(pp_t[:], c_wp1[:], h_pair[:, 1, :],
                                         start=True, stop=True)
                        prelu = mlpp.tile([PROJ, BL], BF16, tag="prelu")
                        nc.vector.tensor_scalar(prelu[:], pp_t[:],
                                                c_bp1[:, 0:1], 0.0,
                                                op0=ALU.add, op1=ALU.max)
                        nc.tensor.matmul(po_t[:, r8 * OUT:(r8 + 1) * OUT],
                                         prelu[:], c_wp2[:],
                                         start=True, stop=True)

                    # ---- gate matmuls for this tick ----
                    if do1:
                        for g in range(4):
                            nc.tensor.matmul(pg_t[:, 1, gsl(g)],
                                             c_whh1[:, gsl(g)],
                                             h_pair[:, 1, :],
                                             start=(g == 0), stop=False)
                        if b1_nonzero:
                            for g in range(4):
                                nc.tensor.matmul(pg_t[:, 1, gsl(g)],
                                                 c_b1[0:1, gsl(g)],
                                                 c_ones1[0:1, :],
                                                 start=False, stop=False)
                    if do0:
                        for g in range(4):
                            nc.tensor.matmul(pg_t[:, 0, gsl(g)],
                                             c_whh0[:, gsl(g)],
                                             h_pair[:, 0, :],
                                             start=False, stop=(g == 3))
                    if do1:
                        for g in range(4):
                            nc.tensor.matmul(pg_t[:, 1, gsl(g)],
                                             c_wih1[:, gsl(g)],
                                             h_pair[:, 0, :],
                                             start=False, stop=(g == 3))
                    # next tick's input-side matmuls (fills PE while
                    # activations/cell math run)
                    if t + 1 <= T:
                        pg_tiles[t + 1] = pgp.tile(
                            [BL, 2, G4], F32, tag="pg", name=f"pg{t + 1}")
                        if t + 1 < T:
                            emit_xg0(t + 1)

                    # ---- activations + cell updates (per layer) ----
                    # ACT order: sig1 (inputs ready early), sig0, sigc1, sigc0
                    if do1:
                        sig1 = cell.tile([BL, G4], FP16, tag="sig1")
                        nc.scalar.activation(sig1[:], pg_t[:, 1, :],
                                             AF.Sigmoid)
                    if do0:
                        sig0 = cell.tile([BL, G4], FP16, tag="sig0")
                        nc.scalar.activation(sig0[:], pg_t[:, 0, :],
                                             AF.Sigmoid)

                    def cell_update(l, sig):
                        # c = f*c + i*tanh(g) ; h/2 = o*(sig(2c)-0.5)
                        t1 = cell.tile([BL, H], FP16, tag=f"t1_{l}")
                        fc = cell.tile([BL, H], F32, tag=f"fc_{l}")
                        sc = cell.tile([BL, H], FP16, tag=f"sc_{l}")
                        nc.vector.scalar_tensor_tensor(
                            t1[:], sig[:, 3 * H:G4], -0.5, sig[:, 0:H],
                            op0=ALU.add, op1=ALU.mult)
                        nc.vector.tensor_mul(fc[:], sig[:, H:2 * H],
                                             c_pair[:, l, :])
                        nc.vector.scalar_tensor_tensor(
                            c_pair[:, l, :], t1[:], 2.0, fc[:],
                            op0=ALU.mult, op1=ALU.add)
                        nc.scalar.activation(sc[:], c_pair[:, l, :],
                                             AF.Sigmoid, scale=2.0)
                        nc.vector.scalar_tensor_tensor(
                            h_pair[:, l, :], sc[:], -0.5,
                            sig[:, 2 * H:3 * H], op0=ALU.add, op1=ALU.mult)

                    if do1:
                        cell_update(1, sig1)
                    if do0:
                        cell_update(0, sig0)

                    if _DEBUG and t == 0:
                        dtmp = cell.tile([BL, G4], F32, tag="dbgpg")
                        nc.vector.tensor_copy(dtmp[:], pg_t[:, 0, :])
                        nc.sync.dma_start(out=dbg_pg.ap(), in_=dtmp[:])
                    if _DEBUG and t == 1:
                        nc.sync.dma_start(out=dbg_h.ap(), in_=h_pair[:])
                        nc.sync.dma_start(out=dbg_c.ap(), in_=c_pair[:])

                    # ---- output stage ----
                    if 0 <= mm and (mm % 8 == 7 or mm == T - 1):
                        n8 = mm % 8 + 1
                        m0 = mm - mm % 8
                        osb = mlpp.tile([BL, 8 * OUT], F32, tag="osb")
                        nc.vector.tensor_add(osb[:, 0:n8 * OUT],
                                             po_t[:, 0:n8 * OUT],
                                             c_bp2[:, 0:n8 * OUT])
                        nc.sync.dma_start(
                            out=out_ap[:, m0 * OUT:(m0 + n8) * OUT],
                            in_=osb[:, 0:n8 * OUT])

                # flush mlp for the last step (mm = T-1)
                for mm in (T - 1,):
                    r8 = mm % 8
                    pp_t = ppp.tile([PROJ, BL], F32, tag="pp")
                    nc.tensor.matmul(pp_t[:], c_wp1[:], h_pair[:, 1, :],
                                     start=True, stop=True)
                    prelu = mlpp.tile([PROJ, BL], BF16, tag="prelu")
                    nc.vector.tensor_scalar(prelu[:], pp_t[:], c_bp1[:, 0:1],
                                            0.0, op0=ALU.add, op1=ALU.max)
                    nc.tensor.matmul(po_t[:, r8 * OUT:(r8 + 1) * OUT],
                                     prelu[:], c_wp2[:], start=True,
                                     stop=True)
                    n8 = r8 + 1
                    m0 = mm - r8
                    osb = mlpp.tile([BL, 8 * OUT], F32, tag="osb")
                    nc.vector.tensor_add(osb[:, 0:n8 * OUT],
                                         po_t[:, 0:n8 * OUT],
                                         c_bp2[:, 0:n8 * OUT])
                    nc.sync.dma_start(out=out_ap[:, m0 * OUT:(m0 + n8) * OUT],
                                      in_=osb[:, 0:n8 * OUT])

    nc.compile()
    return nc


def _get_program(b1_nonzero: bool, v0_nonzero: bool):
    key = (b1_nonzero, v0_nonzero)
    if key not in _COMPILED:
        _COMPILED[key] = _build_program(b1_nonzero, v0_nonzero)
    return _COMPILED[key]


def _prep_host(x, ln_gamma, ln_beta, W_ih0, W_hh0, b0, W_ih1, W_hh1, b1,
               Wp1, bp1, Wp2, bp2):
    f32 = np.float32
    x = np.asarray(x, f32)
    g = np.asarray(ln_gamma, f32)
    be = np.asarray(ln_beta, f32)
    W_ih0 = np.asarray(W_ih0, f32)[_PERM].copy()
    W_hh0 = np.asarray(W_hh0, f32)[_PERM].copy()
    b0 = np.asarray(b0, f32)[_PERM].copy()
    W_ih1 = np.asarray(W_ih1, f32)[_PERM].copy()
    W_hh1 = np.asarray(W_hh1, f32)[_PERM].copy()
    b1 = np.asarray(b1, f32)[_PERM].copy()
    Wp1 = np.asarray(Wp1, f32).copy()
    bp1v = np.asarray(bp1, f32)
    Wp2 = np.asarray(Wp2, f32)
    bp2v = np.asarray(bp2, f32)

    Wt0 = W_ih0 * g[None, :]                  # [512, 158]
    u0 = W_ih0 @ g                            # [512]
    v0 = W_ih0 @ be + b0                      # [512]
    # fold 1: tanh(g)=2*sig(2v)-1 -> double g-gate pre-activation rows
    for M in (Wt0, W_hh0, W_ih1, W_hh1):
        M[_GS] *= 2.0
    u0 = u0.copy()
    u0[_GS] *= 2.0
    v0[_GS] *= 2.0
    b1[_GS] *= 2.0
    # fold 2: h stored halved -> double all consumers of h
    W_hh0 *= 2.0
    W_ih1 *= 2.0
    W_hh1 *= 2.0
    Wp1 *= 2.0

    a_hi = np.ascontiguousarray(Wt0[:, :H].T).astype(NP_BF16)
    # a_lo: rows 0:30 = lo x-weights, row 30 = u (LN fold), row 31 unused
    a_lo4 = np.zeros((KLO, G4), f32)
    a_lo4[0:DLO] = Wt0[:, H:D].T
    a_lo4[DLO] = u0
    a_lo4 = a_lo4.astype(NP_BF16)

    shared = {
        "a_hi": a_hi,
        "a_lo4": a_lo4,
        "whh0": np.ascontiguousarray(W_hh0.T).astype(NP_BF16),
        "wih1": np.ascontiguousarray(W_ih1.T).astype(NP_BF16),
        "whh1": np.ascontiguousarray(W_hh1.T).astype(NP_BF16),
        "wp1": np.ascontiguousarray(Wp1.T).astype(NP_BF16),
        "wp2": np.ascontiguousarray(Wp2.T).astype(NP_BF16),
        "bp1": np.ascontiguousarray(bp1v.reshape(PROJ, 1)),
        "bp2b8": np.ascontiguousarray(
            np.tile(bp2v[None, :], (BL, 8)).astype(f32)),
        "ident": np.eye(H, dtype=f32),
    }
    b1_nonzero = bool(np.any(b1 != 0))
    v0_nonzero = bool(np.any(v0 != 0))
    if b1_nonzero:
        shared["b1r"] = b1.reshape(1, G4).astype(NP_BF16)
    if v0_nonzero:
        shared["v0r"] = v0.reshape(1, G4).astype(NP_BF16)

    in_maps = []
    for c in range(NC_N):
        xc = x[c * BL:(c + 1) * BL]                       # [128, 250, 158]
        xT = np.ascontiguousarray(xc.transpose(2, 1, 0)).reshape(D, T * BL)
        xT16 = xT.astype(NP_BF16)
        xtlo4 = np.zeros((KLO, T * BL), NP_BF16)
        xtlo4[0:DLO] = xT16[H:D]
        m = dict(shared)
        m["xnat"] = np.ascontiguousarray(xc.reshape(BL, T * D))
        m["xthi"] = np.ascontiguousarray(xT16[0:H])
        m["xtlo4"] = xtlo4
        in_maps.append(m)
    return in_maps, b1_nonzero, v0_nonzero


def kernel(**inputs) -> np.ndarray:
    in_maps, b1nz, v0nz = _prep_host(**inputs)
    nc = _get_program(b1nz, v0nz)
    res = run_bass_kernel_spmd(nc, in_maps, core_ids=list(range(NC_N)))
    out = np.empty((B, T, OUT), np.float32)
    for c in range(NC_N):
        out[c * BL:(c + 1) * BL] = res.results[c]["out"].reshape(BL, T, OUT)
    return out
